# revision 25
# baseline (speedup 1.0000x reference)
"""Multi-head attention (B=8, N=1024, C=768, H=12) on 8 Trainium2 NeuronCores.

Sharding: data-parallel, one batch element per core. Each core computes the
full attention block for its batch: QKV projection, per-head softmax(QK^T/8)V,
and the output projection, entirely on-chip (SBUF/PSUM).

Layout strategy (chosen so no on-device transposes are needed):
  - host passes x^T [C, N], w_qkv^T [C, 3C], w_proj^T [C, C], bias replicated
    to [128, C].
  - Q, K are produced transposed ([d, n], head-dim on partitions) by the QKV
    matmul; V is produced in natural [n, d] layout by swapping lhsT/rhs.
  - scores are computed transposed (S^T[m, n] = K Q^T) so that exp(S^T) can be
    consumed directly as the moving operand of the P@V matmul.
  - V tiles carry an appended ones-column, so the P@V matmul's 65th output row
    is the softmax denominator (row-sum of exp scores) for free.
  - normalization multiplies by a reciprocal row broadcast across partitions
    via a DRAM-bounced DMA (SBUF APs cannot partition-broadcast).

Matmul operands are bf16 (single-pass PE streaming, FWL-eligible weight
loads); PSUM accumulation stays fp32.

Scheduling (v4): the exp stream on the ACT engine is the long pole
(~107us of exp for 12.6M scores); everything else fills around it.
  - pair 0's Q/K project first; its scores+exps for both query halves
    emit before anything else so ACT starts at ~15us.
  - each dma_start costs ~0.6us of ISSUE time on its issuing engine, so
    DMA issue is distributed: input loads split between sync and gpsimd,
    the per-(t,j) normalization DMAs run on gpsimd, and the h0/h1
    normalization stages share one tile so one DMA covers both heads.
  - the exp stream runs one j-block ahead of P@V consumption (24 stexp
    buffers); score matmuls are just-in-time producers for it.
  - Q/K projection PSUM groups share the projection-sweep PSUM chain so
    they never gate score-matmul PSUM slots.
  - V projection, later pairs' Q/K, and 2/3 of the output projection run
    inside the ACT-bound window; the projection accumulates pairs 0-3
    into SBUF (yacc) early, leaving only the k=4,5 sweep in the tail,
    interleaved with the last pair's normalization.
  - PSUM-freeing copies (P@V stages, V scatter) run on the otherwise-idle
    gpsimd so they never queue behind DVE work.
"""

import sys

import numpy as np

if "/opt/trn_rl_repo" not in sys.path:
    sys.path.insert(0, "/opt/trn_rl_repo")

B = 8
N = 1024
C = 768
H = 12
D = 64
SCALE = D ** -0.5
KT = C // 128            # 6 contraction tiles over channels
NT = N // 128             # 8 token tiles
PAIRS = H // 2            # 6 head pairs

_CACHE = {}


def build_program(use_bf16=True):
    import concourse.bacc as bacc
    import concourse.mybir as mybir
    import concourse.tile as tile

    f32 = mybir.dt.float32
    f32r = mybir.dt.float32r
    Exp = mybir.ActivationFunctionType.Exp
    fm = mybir.dt.bfloat16 if use_bf16 else mybir.dt.float32r

    nc = bacc.Bacc("TRN2", target_bir_lowering=False, debug=False)

    xT_d = nc.dram_tensor("xT", [C, N], fm, kind="ExternalInput")
    # Q/K weights repacked pair-major on host: pair t = [wq_t | wk_t] 256 cols
    wqkpT_d = nc.dram_tensor("wqkpT", [C, PAIRS * 256], fm, kind="ExternalInput")
    wqkvT_d = nc.dram_tensor("wqkvT", [C, 3 * C], fm, kind="ExternalInput")
    wprojT_d = nc.dram_tensor("wprojT", [C, C], fm, kind="ExternalInput")
    bias_d = nc.dram_tensor("bias_rep", [128, C], f32, kind="ExternalInput")
    y_d = nc.dram_tensor("y", [N, C], f32, kind="ExternalOutput")

    mm = nc.tensor.matmul

    with tile.TileContext(nc) as tc:
        with tc.tile_pool(name="pers", bufs=1) as pers, \
             tc.tile_pool(name="qa", bufs=13) as qa, \
             tc.tile_pool(name="stp", bufs=24) as stp, \
             tc.tile_pool(name="cyc", bufs=2) as pB, \
             tc.tile_pool(name="dramb", bufs=2, space="DRAM") as pDr, \
             tc.tile_pool(name="ps_s", bufs=2, space="PSUM") as psS, \
             tc.tile_pool(name="ps_y", bufs=2, space="PSUM") as psY, \
             tc.tile_pool(name="ps_p", bufs=2, space="PSUM") as psP:
            # Q^T,K^T tiles [d, n]: tile m holds heads 2m (parts 0:64) and
            # 2m+1 (parts 64:128); m 0..5 = Q, 6..11 = K. aot (attn out^T)
            # shares the 13-slot tag chain, reusing dead Q/K slots.
            qkt = [None] * (2 * PAIRS)
            # V tiles [n-tile, pair, 130]: per pair block [V_h0 |1| V_h1 |1];
            # ones cols at 64 and 129 feed the denominator row of P@V.
            vbuf = [pers.tile([128, PAIRS, 130], fm, name=f"vbuf{i}", tag=f"vbuf{i}")
                    for i in range(NT)]
            xt = [pers.tile([128, N], fm, name=f"xt{k}", tag=f"xt{k}")
                  for k in range(KT)]
            wqkp = [pers.tile([128, KT, 256], fm, name=f"wqkp{t}", tag=f"wqkp{t}")
                    for t in range(PAIRS)]
            wv = pers.tile([128, KT, C], fm, name="wv", tag="wv")
            wp = pers.tile([128, KT, C], fm, name="wp", tag="wp")
            bias_t = pers.tile([128, C], f32, name="bias_t", tag="bias_t")
            yacc = [pers.tile([128, C], f32, name=f"yacc{i}", tag=f"yacc{i}")
                    for i in range(NT)]

            # input loads: each dma_start costs ~0.6us of issue time on its
            # engine, so loads are few and pair-0's weights come first.
            # sync: pair-0 Q/K weights, then x (per-k so the projection's
            # k-accumulation chases arrivals), then pair 1.
            def dma_wqkp(t, eng):
                eng.dma_start(
                    wqkp[t][:],
                    wqkpT_d[:, 256 * t:256 * (t + 1)].rearrange(
                        "(k p) c -> p k c", p=128))
            dma_wqkp(0, nc.sync)
            for k in range(KT):
                nc.sync.dma_start(xt[k][:], xT_d[128 * k:128 * (k + 1), :])
            dma_wqkp(1, nc.sync)
            nc.gpsimd.dma_start(
                wv[:], wqkvT_d[:, 2 * C:3 * C].rearrange("(k p) c -> p k c",
                                                         p=128))
            for t in range(2, PAIRS):
                dma_wqkp(t, nc.gpsimd)
            nc.gpsimd.dma_start(
                wp[:], wprojT_d[:, :].rearrange("(k p) c -> p k c", p=128))
            nc.gpsimd.dma_start(bias_t[:], bias_d[:])
            for i in range(NT):
                ones_ap = vbuf[i].rearrange("p a (t c) -> p a t c", c=65)[:, :, :, 64]
                nc.vector.memset(ones_ap, 1.0)
            # rank-1 broadcast stationary for the softmax normalization;
            # row 64 so its base partition matches the denominator row's
            ones_r = pers.tile([65, 64], f32r, name="ones_r", tag="ones_r")
            nc.vector.memset(ones_r[64:65, :].bitcast(f32), 1.0)

            def emit_qk(m):
                t_ = qa.tile([128, N], fm, name=f"qkt{m}", tag="qa")
                qkt[m] = t_
                tp, o0 = (m, 0) if m < PAIRS else (m - PAIRS, 128)
                for j in range(2):
                    ps = psP.tile([128, 512], f32, name="qk_ps", tag="pp")
                    for k in range(KT):
                        mm(ps[:], wqkp[tp][:, k, o0:o0 + 128],
                           xt[k][:, 512 * j:512 * (j + 1)],
                           start=(k == 0), stop=(k == KT - 1))
                    nc.vector.tensor_copy(t_[:, 512 * j:512 * (j + 1)], ps[:])

            def emit_v(i):
                for c0, w in ((0, 512), (512, 256)):
                    ps = psY.tile([128, 512], f32, name="v_ps", tag="py")
                    for k in range(KT):
                        mm(ps[:, 0:w], xt[k][:, 128 * i:128 * (i + 1)],
                           wv[:, k, c0:c0 + w],
                           start=(k == 0), stop=(k == KT - 1))
                    # scatter heads: even -> cols 0:64, odd -> cols 65:129
                    # within each 130-wide pair block
                    v_view = ps[:, 0:w].rearrange("p (a t c) -> p a t c",
                                                  t=2, c=64)
                    pa0 = c0 // 128
                    npair = w // 128
                    nc.vector.tensor_copy(
                        vbuf[i][:, pa0:pa0 + npair, 0:64], v_view[:, :, 0, :])
                    nc.vector.tensor_copy(
                        vbuf[i][:, pa0:pa0 + npair, 65:129], v_view[:, :, 1, :])

            # scores + exp for (pair t, query half j): feeds the ACT stream
            stexps = {}

            def emit_scores(t, j):
                qt, kt = qkt[t], qkt[PAIRS + t]
                lst = []
                for i in range(NT):
                    stexp = stp.tile([128, 2, 512], fm, name="stexp",
                                     tag="stexp")
                    s_ps = psS.tile([128, 1024], f32, name="s_ps", tag="ps")
                    for h in range(2):
                        # S^T[m, n] = sum_d K^T[d, m] Q^T[d, n]; h0/h1 use
                        # distinct PE row groups (base partition 0 / 64).
                        mm(s_ps[:, 512 * h:512 * (h + 1)],
                           kt[64 * h:64 * (h + 1), 128 * i:128 * (i + 1)],
                           qt[64 * h:64 * (h + 1), 512 * j:512 * (j + 1)],
                           start=True, stop=True)
                    # exp(S^T / 8) for both heads, PSUM -> SBUF bf16
                    nc.scalar.activation(
                        stexp[:, :, :],
                        s_ps[:].rearrange("p (h n) -> p h n", h=2),
                        Exp, scale=SCALE)
                    lst.append(stexp)
                stexps[(t, j)] = lst

            # P@V + normalization for (pair t, query half j)
            def emit_pv(t, j, aot):
                lst = stexps.pop((t, j))
                pv_ps = [psY.tile([65, 512], f32, name=f"pv{h}", tag="py")
                         for h in range(2)]
                for i in range(NT):
                    for h in range(2):
                        # rows 0:64 = (P~ @ V)^T, row 64 = denominator
                        mm(pv_ps[h][:],
                           vbuf[i][:, t, 65 * h:65 * (h + 1)],
                           lst[i][:, h, :],
                           start=(i == 0), stop=(i == NT - 1))

                # normalization: DVE copies free the P@V banks; the
                # reciprocal runs in place on the [1, 1024] denominator row
                # and a rank-1 f32r matmul (ones x recip-row) broadcasts it
                # across 64 partitions -- no DMA hops at all.
                stage = pB.tile([65, 2, 512], f32, name="stage", tag="stage")
                for h in range(2):
                    nc.vector.tensor_copy(stage[:, h, :], pv_ps[h][:])
                denrow = pB.tile([65, 2, 512], f32r, name="denrow",
                                 tag="denrow")
                with nc.allow_low_precision(reason="f32r reciprocal row"):
                    nc.vector.reciprocal(denrow[64:65, :, :],
                                         stage[64:65, :, :])
                rbs = []
                for h in range(2):
                    rb = psP.tile([64, 512], f32, name="rb", tag="pp")
                    mm(rb[:], ones_r[64:65, :], denrow[64:65, h, :],
                       start=True, stop=True)
                    rbs.append(rb)
                nc.vector.tensor_mul(
                    aot[0:64, 512 * j:512 * (j + 1)],
                    stage[0:64, 0, :], rbs[0][:])
                tmp = pB.tile([64, 512], fm, name="tmp1", tag="tmp1")
                nc.vector.tensor_mul(tmp[:], stage[0:64, 1, :], rbs[1][:])
                # DVE lanes cannot shift partitions; DMA moves the odd head
                # into partitions 64:128.
                nc.gpsimd.dma_start(
                    aot[64:128, 512 * j:512 * (j + 1)], tmp[:])

            # output projection sweep over pairs k0..k1 for token tiles isl;
            # k<4 accumulates bias+partials into yacc, k>=4 finishes into yt
            def emit_proj(k0, k1, isl, aot_all):
                for i in isl:
                    for c0 in (0, 384):
                        pp = psP.tile([128, 512], f32, name="pp", tag="pp")
                        for k in range(k0, k1):
                            mm(pp[:, 0:384],
                               aot_all[k][:, 128 * i:128 * (i + 1)],
                               wp[:, k, c0:c0 + 384],
                               start=(k == k0), stop=(k == k1 - 1))
                        if k0 == 0:
                            nc.vector.tensor_add(yacc[i][:, c0:c0 + 384],
                                                 pp[:, 0:384],
                                                 bias_t[:, c0:c0 + 384])
                        else:
                            nc.vector.tensor_add(yacc[i][:, c0:c0 + 384],
                                                 pp[:, 0:384],
                                                 yacc[i][:, c0:c0 + 384])
                    if k0 != 0:
                        nc.gpsimd.dma_start(y_d[128 * i:128 * (i + 1), :],
                                            yacc[i][:])

            # ---- emission schedule ----
            aot_all = []

            def new_aot(t):
                a = qa.tile([128, N], fm, name=f"aot{t}", tag="qa")
                aot_all.append(a)
                return a

            emit_qk(0)
            emit_qk(PAIRS)
            emit_scores(0, 0)            # ACT stream starts here
            emit_scores(0, 1)
            emit_qk(1)
            emit_qk(PAIRS + 1)
            emit_scores(1, 0)            # one j-block of exp-ahead
            for i in range(NT):
                emit_v(i)
            aot0 = new_aot(0)
            emit_pv(0, 0, aot0)
            emit_qk(2)
            emit_qk(PAIRS + 2)
            emit_scores(1, 1)
            emit_pv(0, 1, aot0)

            for t in range(1, PAIRS):
                aot = new_aot(t)
                emit_pv(t, 0, aot)
                if t + 2 < PAIRS:
                    emit_qk(t + 2)
                    emit_qk(PAIRS + t + 2)
                if t >= 2:
                    emit_scores(t, 1)   # (1,1) was already emitted pre-loop
                if t + 1 < PAIRS:
                    emit_scores(t + 1, 0)
                if t == 5:
                    emit_proj(4, KT, range(0, 4), aot_all)
                emit_pv(t, 1, aot)
                if t == 3:
                    emit_proj(0, 4, range(0, 4), aot_all)
                elif t == 4:
                    emit_proj(0, 4, range(4, NT), aot_all)
            emit_proj(4, KT, range(4, NT), aot_all)

    nc.compile()
    return nc


def round_f32r(a):
    """Round fp32 to the FP32r grid (11 explicit mantissa bits, RNE)."""
    a = np.ascontiguousarray(a, dtype=np.float32)
    b = a.view(np.uint32)
    r = (b + np.uint32(0x7FF) + ((b >> np.uint32(12)) & np.uint32(1))) \
        & np.uint32(0xFFFFF000)
    return r.view(np.float32)


USE_BF16 = True


def make_in_maps(x, w_qkv, w_proj, b_proj):
    if USE_BF16:
        import ml_dtypes
        cvt = lambda a: np.ascontiguousarray(a).astype(ml_dtypes.bfloat16)
    else:
        cvt = round_f32r
    wqkvT_f = np.asarray(w_qkv, dtype=np.float32).T
    wqkpT = np.concatenate(
        [np.concatenate([wqkvT_f[:, 128 * t:128 * (t + 1)],
                         wqkvT_f[:, C + 128 * t:C + 128 * (t + 1)]], axis=1)
         for t in range(PAIRS)], axis=1)
    wqkpT = cvt(wqkpT)
    wqkvT = cvt(wqkvT_f)
    wprojT = cvt(np.asarray(w_proj, dtype=np.float32).T)
    bias_rep = np.ascontiguousarray(
        np.broadcast_to(np.asarray(b_proj, dtype=np.float32), (128, C)))
    x = np.asarray(x, dtype=np.float32)
    return [
        {
            "xT": cvt(x[b].T),
            "wqkpT": wqkpT,
            "wqkvT": wqkvT,
            "wprojT": wprojT,
            "bias_rep": bias_rep,
        }
        for b in range(B)
    ]


def kernel(x, w_qkv, w_proj, b_proj):
    from concourse.bass_utils import run_bass_kernel_spmd

    if "nc" not in _CACHE:
        _CACHE["nc"] = build_program(use_bf16=USE_BF16)
    nc = _CACHE["nc"]

    in_maps = make_in_maps(x, w_qkv, w_proj, b_proj)
    res = run_bass_kernel_spmd(nc, in_maps, core_ids=list(range(B)))
    out = np.stack([res.results[b]["y"] for b in range(B)], axis=0)
    return out.astype(np.float32)


# revision 27
# speedup vs baseline: 1.3708x; 1.3708x over previous
"""Multi-head attention (B=8, N=1024, C=768, H=12) on 8 Trainium2 NeuronCores.

Sharding: data-parallel, one batch element per core. Each core computes the
full attention block for its batch: QKV projection, per-head softmax(QK^T/8)V,
and the output projection, entirely on-chip (SBUF/PSUM).

Layout strategy (chosen so no on-device transposes are needed):
  - host passes x^T [C, N], w_qkv^T [C, 3C], w_proj^T [C, C], bias replicated
    to [128, C].
  - Q, K are produced transposed ([d, n], head-dim on partitions) by the QKV
    matmul; V is produced in natural [n, d] layout by swapping lhsT/rhs.
  - scores are computed transposed (S^T[m, n] = K Q^T) so that exp(S^T) can be
    consumed directly as the moving operand of the P@V matmul.
  - V tiles carry an appended ones-column, so the P@V matmul's 65th output row
    is the softmax denominator (row-sum of exp scores) for free.
  - normalization multiplies by a reciprocal row broadcast across partitions
    via a DRAM-bounced DMA (SBUF APs cannot partition-broadcast).

Matmul operands are bf16 (single-pass PE streaming, FWL-eligible weight
loads); PSUM accumulation stays fp32.

Scheduling (v4): the exp stream on the ACT engine is the long pole
(~107us of exp for 12.6M scores); everything else fills around it.
  - pair 0's Q/K project first; its scores+exps for both query halves
    emit before anything else so ACT starts at ~15us.
  - each dma_start costs ~0.6us of ISSUE time on its issuing engine, so
    DMA issue is distributed: input loads split between sync and gpsimd,
    the per-(t,j) normalization DMAs run on gpsimd, and the h0/h1
    normalization stages share one tile so one DMA covers both heads.
  - the exp stream runs one j-block ahead of P@V consumption (24 stexp
    buffers); score matmuls are just-in-time producers for it.
  - Q/K projection PSUM groups share the projection-sweep PSUM chain so
    they never gate score-matmul PSUM slots.
  - V projection, later pairs' Q/K, and 2/3 of the output projection run
    inside the ACT-bound window; the projection accumulates pairs 0-3
    into SBUF (yacc) early, leaving only the k=4,5 sweep in the tail,
    interleaved with the last pair's normalization.
  - PSUM-freeing copies (P@V stages, V scatter) run on the otherwise-idle
    gpsimd so they never queue behind DVE work.
"""

import sys

import numpy as np

if "/opt/trn_rl_repo" not in sys.path:
    sys.path.insert(0, "/opt/trn_rl_repo")

B = 8
N = 1024
C = 768
H = 12
D = 64
SCALE = D ** -0.5
KT = C // 128            # 6 contraction tiles over channels
NT = N // 128             # 8 token tiles
PAIRS = H // 2            # 6 head pairs

_CACHE = {}


def build_program(use_bf16=True):
    import concourse.bacc as bacc
    import concourse.mybir as mybir
    import concourse.tile as tile

    f32 = mybir.dt.float32
    f32r = mybir.dt.float32r
    Exp = mybir.ActivationFunctionType.Exp
    fm = mybir.dt.bfloat16 if use_bf16 else mybir.dt.float32r

    nc = bacc.Bacc("TRN2", target_bir_lowering=False, debug=False)

    xT_d = nc.dram_tensor("xT", [C, N], fm, kind="ExternalInput")
    # Q/K weights repacked pair-major on host: pair t = [wq_t | wk_t] 256 cols
    wqkpT_d = nc.dram_tensor("wqkpT", [C, PAIRS * 256], fm, kind="ExternalInput")
    wqkvT_d = nc.dram_tensor("wqkvT", [C, 3 * C], fm, kind="ExternalInput")
    wprojT_d = nc.dram_tensor("wprojT", [C, C], fm, kind="ExternalInput")
    bias_d = nc.dram_tensor("bias_rep", [128, C], f32, kind="ExternalInput")
    y_d = nc.dram_tensor("y", [N, C], f32, kind="ExternalOutput")

    mm = nc.tensor.matmul

    with tile.TileContext(nc) as tc:
        with tc.tile_pool(name="pers", bufs=1) as pers, \
             tc.tile_pool(name="qa", bufs=13) as qa, \
             tc.tile_pool(name="stp", bufs=24) as stp, \
             tc.tile_pool(name="cyc", bufs=2) as pB, \
             tc.tile_pool(name="dramb", bufs=2, space="DRAM") as pDr, \
             tc.tile_pool(name="ps_s", bufs=2, space="PSUM") as psS, \
             tc.tile_pool(name="ps_y", bufs=2, space="PSUM") as psY, \
             tc.tile_pool(name="ps_p", bufs=2, space="PSUM") as psP:
            # Q^T,K^T tiles [d, n]: tile m holds heads 2m (parts 0:64) and
            # 2m+1 (parts 64:128); m 0..5 = Q, 6..11 = K. aot (attn out^T)
            # shares the 13-slot tag chain, reusing dead Q/K slots.
            qkt = [None] * (2 * PAIRS)
            # V tiles [n-tile, pair, 130]: per pair block [V_h0 |1| V_h1 |1];
            # ones cols at 64 and 129 feed the denominator row of P@V.
            vbuf = [pers.tile([128, PAIRS, 130], fm, name=f"vbuf{i}", tag=f"vbuf{i}")
                    for i in range(NT)]
            xt = [pers.tile([128, N], fm, name=f"xt{k}", tag=f"xt{k}")
                  for k in range(KT)]
            wqkp = [pers.tile([128, KT, 256], fm, name=f"wqkp{t}", tag=f"wqkp{t}")
                    for t in range(PAIRS)]
            wv = pers.tile([128, KT, C], fm, name="wv", tag="wv")
            wp = pers.tile([128, KT, C], fm, name="wp", tag="wp")
            bias_t = pers.tile([128, C], f32, name="bias_t", tag="bias_t")
            yacc = [pers.tile([128, C], f32, name=f"yacc{i}", tag=f"yacc{i}")
                    for i in range(NT)]

            # input loads: each dma_start costs ~0.6us of issue time on its
            # engine, so loads are few and pair-0's weights come first.
            # sync: pair-0 Q/K weights, then x (per-k so the projection's
            # k-accumulation chases arrivals), then pair 1.
            def dma_wqkp(t, eng):
                eng.dma_start(
                    wqkp[t][:],
                    wqkpT_d[:, 256 * t:256 * (t + 1)].rearrange(
                        "(k p) c -> p k c", p=128))
            dma_wqkp(0, nc.sync)
            for k in range(KT):
                nc.sync.dma_start(xt[k][:], xT_d[128 * k:128 * (k + 1), :])
            dma_wqkp(1, nc.sync)
            nc.gpsimd.dma_start(
                wv[:], wqkvT_d[:, 2 * C:3 * C].rearrange("(k p) c -> p k c",
                                                         p=128))
            for t in range(2, PAIRS):
                dma_wqkp(t, nc.gpsimd)
            nc.gpsimd.dma_start(
                wp[:], wprojT_d[:, :].rearrange("(k p) c -> p k c", p=128))
            nc.gpsimd.dma_start(bias_t[:], bias_d[:])
            for i in range(NT):
                ones_ap = vbuf[i].rearrange("p a (t c) -> p a t c", c=65)[:, :, :, 64]
                nc.vector.memset(ones_ap, 1.0)
            # rank-1 broadcast stationary for the softmax normalization
            ones_r = pers.tile([1, 64], f32r, name="ones_r", tag="ones_r")
            nc.vector.memset(ones_r[:].bitcast(f32), 1.0)

            def emit_qk(m):
                t_ = qa.tile([128, N], fm, name=f"qkt{m}", tag="qa")
                qkt[m] = t_
                tp, o0 = (m, 0) if m < PAIRS else (m - PAIRS, 128)
                for j in range(2):
                    ps = psP.tile([128, 512], f32, name="qk_ps", tag="pp")
                    for k in range(KT):
                        mm(ps[:], wqkp[tp][:, k, o0:o0 + 128],
                           xt[k][:, 512 * j:512 * (j + 1)],
                           start=(k == 0), stop=(k == KT - 1))
                    nc.vector.tensor_copy(t_[:, 512 * j:512 * (j + 1)], ps[:])

            def emit_v(i):
                for c0, w in ((0, 512), (512, 256)):
                    ps = psY.tile([128, 512], f32, name="v_ps", tag="py")
                    for k in range(KT):
                        mm(ps[:, 0:w], xt[k][:, 128 * i:128 * (i + 1)],
                           wv[:, k, c0:c0 + w],
                           start=(k == 0), stop=(k == KT - 1))
                    # scatter heads: even -> cols 0:64, odd -> cols 65:129
                    # within each 130-wide pair block
                    v_view = ps[:, 0:w].rearrange("p (a t c) -> p a t c",
                                                  t=2, c=64)
                    pa0 = c0 // 128
                    npair = w // 128
                    nc.vector.tensor_copy(
                        vbuf[i][:, pa0:pa0 + npair, 0:64], v_view[:, :, 0, :])
                    nc.vector.tensor_copy(
                        vbuf[i][:, pa0:pa0 + npair, 65:129], v_view[:, :, 1, :])

            # scores + exp for (pair t, query half j): feeds the ACT stream
            stexps = {}

            def emit_scores(t, j):
                qt, kt = qkt[t], qkt[PAIRS + t]
                lst = []
                for i in range(NT):
                    stexp = stp.tile([128, 2, 512], fm, name="stexp",
                                     tag="stexp")
                    s_ps = psS.tile([128, 1024], f32, name="s_ps", tag="ps")
                    for h in range(2):
                        # S^T[m, n] = sum_d K^T[d, m] Q^T[d, n]; h0/h1 use
                        # distinct PE row groups (base partition 0 / 64).
                        mm(s_ps[:, 512 * h:512 * (h + 1)],
                           kt[64 * h:64 * (h + 1), 128 * i:128 * (i + 1)],
                           qt[64 * h:64 * (h + 1), 512 * j:512 * (j + 1)],
                           start=True, stop=True)
                    # exp(S^T / 8) for both heads, PSUM -> SBUF bf16
                    nc.scalar.activation(
                        stexp[:, :, :],
                        s_ps[:].rearrange("p (h n) -> p h n", h=2),
                        Exp, scale=SCALE)
                    lst.append(stexp)
                stexps[(t, j)] = lst

            # P@V + normalization for (pair t, query half j)
            def emit_pv(t, j, aot):
                lst = stexps.pop((t, j))
                pv_ps = [psY.tile([65, 512], f32, name=f"pv{h}", tag="py")
                         for h in range(2)]
                for i in range(NT):
                    for h in range(2):
                        # rows 0:64 = (P~ @ V)^T, row 64 = denominator
                        mm(pv_ps[h][:],
                           vbuf[i][:, t, 65 * h:65 * (h + 1)],
                           lst[i][:, h, :],
                           start=(i == 0), stop=(i == NT - 1))

                # normalization: DVE copies free the P@V banks; the
                # reciprocal runs in place on the [1, 1024] denominator row
                # and a rank-1 f32r matmul (ones x recip-row) broadcasts it
                # across 64 partitions -- no DMA hops at all.
                stage = pB.tile([65, 2, 512], f32, name="stage", tag="stage")
                for h in range(2):
                    nc.vector.tensor_copy(stage[:, h, :], pv_ps[h][:])
                # [1, 1024] DVE reciprocal is FD-bound (~6.5us); DMA the
                # denominator rows into [128, 8] first where it's ~130ns,
                # then bring the reciprocal row back for the rank-1
                # broadcast matmul.
                den_t = pB.tile([128, 8], f32, name="den_t", tag="den_t")
                nc.gpsimd.dma_start(den_t[:], stage[64:65, :, :])
                den_r = pB.tile([128, 8], f32r, name="den_r", tag="den_r")
                nc.vector.reciprocal(den_r[:].bitcast(f32), den_t[:])
                dr2 = pB.tile([1, 1024], f32r, name="dr2", tag="dr2")
                nc.gpsimd.dma_start(
                    dr2[:].rearrange("p (a b) -> p a b", a=128), den_r[:])
                rbs = []
                for h in range(2):
                    rb = psP.tile([64, 512], f32, name="rb", tag="pp")
                    mm(rb[:], ones_r[:], dr2[:, 512 * h:512 * (h + 1)],
                       start=True, stop=True)
                    rbs.append(rb)
                nc.vector.tensor_mul(
                    aot[0:64, 512 * j:512 * (j + 1)],
                    stage[0:64, 0, :], rbs[0][:])
                tmp = pB.tile([64, 512], fm, name="tmp1", tag="tmp1")
                nc.vector.tensor_mul(tmp[:], stage[0:64, 1, :], rbs[1][:])
                # DVE lanes cannot shift partitions; DMA moves the odd head
                # into partitions 64:128.
                nc.gpsimd.dma_start(
                    aot[64:128, 512 * j:512 * (j + 1)], tmp[:])

            # output projection sweep over pairs k0..k1 for token tiles isl;
            # k<4 accumulates bias+partials into yacc, k>=4 finishes into yt
            def emit_proj(k0, k1, isl, aot_all):
                for i in isl:
                    for c0 in (0, 384):
                        pp = psP.tile([128, 512], f32, name="pp", tag="pp")
                        for k in range(k0, k1):
                            mm(pp[:, 0:384],
                               aot_all[k][:, 128 * i:128 * (i + 1)],
                               wp[:, k, c0:c0 + 384],
                               start=(k == k0), stop=(k == k1 - 1))
                        if k0 == 0:
                            nc.vector.tensor_add(yacc[i][:, c0:c0 + 384],
                                                 pp[:, 0:384],
                                                 bias_t[:, c0:c0 + 384])
                        else:
                            nc.vector.tensor_add(yacc[i][:, c0:c0 + 384],
                                                 pp[:, 0:384],
                                                 yacc[i][:, c0:c0 + 384])
                    if k0 != 0:
                        nc.gpsimd.dma_start(y_d[128 * i:128 * (i + 1), :],
                                            yacc[i][:])

            # ---- emission schedule ----
            aot_all = []

            def new_aot(t):
                a = qa.tile([128, N], fm, name=f"aot{t}", tag="qa")
                aot_all.append(a)
                return a

            emit_qk(0)
            emit_qk(PAIRS)
            emit_scores(0, 0)            # ACT stream starts here
            emit_scores(0, 1)
            emit_qk(1)
            emit_qk(PAIRS + 1)
            emit_scores(1, 0)            # one j-block of exp-ahead
            for i in range(NT):
                emit_v(i)
            aot0 = new_aot(0)
            emit_pv(0, 0, aot0)
            emit_qk(2)
            emit_qk(PAIRS + 2)
            emit_scores(1, 1)
            emit_pv(0, 1, aot0)

            for t in range(1, PAIRS):
                aot = new_aot(t)
                emit_pv(t, 0, aot)
                if t + 2 < PAIRS:
                    emit_qk(t + 2)
                    emit_qk(PAIRS + t + 2)
                if t >= 2:
                    emit_scores(t, 1)   # (1,1) was already emitted pre-loop
                if t + 1 < PAIRS:
                    emit_scores(t + 1, 0)
                if t == 5:
                    emit_proj(4, KT, range(0, 4), aot_all)
                emit_pv(t, 1, aot)
                if t == 3:
                    emit_proj(0, 4, range(0, 4), aot_all)
                elif t == 4:
                    emit_proj(0, 4, range(4, NT), aot_all)
            emit_proj(4, KT, range(4, NT), aot_all)

    nc.compile()
    return nc


def round_f32r(a):
    """Round fp32 to the FP32r grid (11 explicit mantissa bits, RNE)."""
    a = np.ascontiguousarray(a, dtype=np.float32)
    b = a.view(np.uint32)
    r = (b + np.uint32(0x7FF) + ((b >> np.uint32(12)) & np.uint32(1))) \
        & np.uint32(0xFFFFF000)
    return r.view(np.float32)


USE_BF16 = True


def make_in_maps(x, w_qkv, w_proj, b_proj):
    if USE_BF16:
        import ml_dtypes
        cvt = lambda a: np.ascontiguousarray(a).astype(ml_dtypes.bfloat16)
    else:
        cvt = round_f32r
    wqkvT_f = np.asarray(w_qkv, dtype=np.float32).T
    wqkpT = np.concatenate(
        [np.concatenate([wqkvT_f[:, 128 * t:128 * (t + 1)],
                         wqkvT_f[:, C + 128 * t:C + 128 * (t + 1)]], axis=1)
         for t in range(PAIRS)], axis=1)
    wqkpT = cvt(wqkpT)
    wqkvT = cvt(wqkvT_f)
    wprojT = cvt(np.asarray(w_proj, dtype=np.float32).T)
    bias_rep = np.ascontiguousarray(
        np.broadcast_to(np.asarray(b_proj, dtype=np.float32), (128, C)))
    x = np.asarray(x, dtype=np.float32)
    return [
        {
            "xT": cvt(x[b].T),
            "wqkpT": wqkpT,
            "wqkvT": wqkvT,
            "wprojT": wprojT,
            "bias_rep": bias_rep,
        }
        for b in range(B)
    ]


def kernel(x, w_qkv, w_proj, b_proj):
    from concourse.bass_utils import run_bass_kernel_spmd

    if "nc" not in _CACHE:
        _CACHE["nc"] = build_program(use_bf16=USE_BF16)
    nc = _CACHE["nc"]

    in_maps = make_in_maps(x, w_qkv, w_proj, b_proj)
    res = run_bass_kernel_spmd(nc, in_maps, core_ids=list(range(B)))
    out = np.stack([res.results[b]["y"] for b in range(B)], axis=0)
    return out.astype(np.float32)
